# revision 8
# baseline (speedup 1.0000x reference)
"""Trainium2 Bass kernel for nn_BinaryTreeLogicNet.

Computes, for x:[B,256], W_leaf:[256,256], weights:[255,2], biases:[255],
w_out:[1,1], b_out:[1]:

    leaf = sigmoid(x @ W_leaf.T - 2)                       # (B, 256)
    8-level pairwise tree reduce with generalized-gcd nodes # (B, 1)
    out  = sigmoid(root * w_out + b_out)

Key algebraic simplifications (exact up to ~1e-7 absolute):
  - every tree value is positive (sigmoid outputs x positive weights), so
    |.| is a no-op and min/max are plain min/max;
  - lam*min + (1-lam)*max = a*(l+r) + (b-a)*max(l,r) with a = lam, b = 1-lam;
  - each node's output is consumed exactly once, scaled by one weight of the
    next level (w_out at the root), so those weights fold into per-node
    constants A = k*lam, C = k*(1-2lam) computed on the host;
  - the reference's +EPS on both children folds to "+k*EPS after the combine",
    implemented as A*(s + delta) with a per-level immediate delta ~ EPS/lam.

Sharding: pure data parallel over the batch dim across 8 cores. x is
transposed on the host so the contraction dim lands on SBUF partitions and
all DMA is contiguous.
"""

import numpy as np

import concourse.bass as bass
import concourse.bacc as bacc
import concourse.mybir as mybir
import concourse.tile as tile
from concourse.bass_utils import run_bass_kernel_spmd

# ---- problem geometry (hardcoded per contract) ----
B, L = 65536, 256
N_CORES = 8
BS = B // N_CORES            # 8192 rows per core
TILES = BS // 128            # 64 tiles of 128 rows
GROUPS = 4
T = TILES // GROUPS          # 16 tiles per group
GCOLS = T * 128              # 2048 batch columns per group

EPS = 1e-6
SHARPNESS = 1.0
BIAS_SHIFT = -2.0

# dtypes (tunable): matmul path and tree path
MM_DT = mybir.dt.float32r    # fp32 bits, fast PE path (1 cyc/row at N>=256)
TREE_DT = mybir.dt.float32

# engine assignment per tree op: 'v' = vector(DVE), 'g' = gpsimd
# ops: s = l+r, mx = max(l,r), q1 = (s+delta)*A, q2 = mx*C, nd = q1+q2
ENG_PLAN = {
    # level index 0..7 (m = 128 >> li)
    li: {"s": "v", "mx": "v", "q1": "v", "q2": "v", "nd": "v"}
    for li in range(8)
}
WINT_ENG = "v"


def _sigmoid(z):
    return 1.0 / (1.0 + np.exp(-z))


def _levels():
    """[(offset, m)] for m = 128, 64, ..., 1 into the weights/biases arrays."""
    out, off, m = [], 0, 128
    while m >= 1:
        out.append((off, m))
        off += m
        m //= 2
    return out


def prep_consts(weights, biases, w_out):
    """Host-folded per-node constants.

    Returns (wint[256], A_cat[255], C_cat[255], deltas[8]) in float32,
    where A/C are concatenated level-by-level (sizes 128, 64, ..., 1).
    """
    w = weights.astype(np.float64)
    b = biases.astype(np.float64)
    lv = _levels()
    A_parts, C_parts, deltas = [], [], []
    for li, (off, m) in enumerate(lv):
        lam = _sigmoid(b[off : off + m])
        if li + 1 < len(lv):
            noff, nm = lv[li + 1]
            k = np.empty(m, np.float64)
            k[0::2] = w[noff : noff + nm, 0]
            k[1::2] = w[noff : noff + nm, 1]
        else:
            k = np.full(m, float(w_out[0, 0]), np.float64)
        A_parts.append(lam * k)
        C_parts.append(k * (1.0 - 2.0 * lam))
        deltas.append(float(EPS * np.mean(1.0 / lam)))
    wint = np.empty(256, np.float64)
    wint[0::2] = w[0:128, 0]
    wint[1::2] = w[0:128, 1]
    return (
        wint.astype(np.float32),
        np.concatenate(A_parts).astype(np.float32),
        np.concatenate(C_parts).astype(np.float32),
        deltas,
    )


def host_emulate(x, W_leaf, weights, biases, w_out, b_out, dtype=np.float32):
    """Pure-numpy emulation of the exact kernel math (for validation)."""
    wint, A_cat, C_cat, deltas = prep_consts(weights, biases, w_out)
    leaf = _sigmoid(
        (x.astype(dtype) @ W_leaf.T.astype(dtype)).astype(dtype) + BIAS_SHIFT
    ).astype(dtype)
    cur = (leaf * wint.astype(dtype)).astype(dtype)
    off = 0
    for li, (_, m) in enumerate(_levels()):
        l_, r_ = cur[:, 0::2], cur[:, 1::2]
        s = (l_ + r_).astype(dtype)
        mx = np.maximum(l_, r_)
        A = A_cat[off : off + m].astype(dtype)
        C = C_cat[off : off + m].astype(dtype)
        cur = (((s + dtype(deltas[li])) * A).astype(dtype) + (mx * C).astype(dtype)).astype(dtype)
        off += m
    return _sigmoid(cur.astype(np.float32) + np.float32(b_out[0]))


def build_nc(b_out_val, deltas):
    """Build the per-core Bass program (SPMD; same NEFF on all cores)."""
    nc = bacc.Bacc("TRN2", target_bir_lowering=False, debug=False)

    xt = nc.dram_tensor("xt", [2, 128, BS], MM_DT, kind="ExternalInput")
    wt = nc.dram_tensor("wt", [128, 2, 256], MM_DT, kind="ExternalInput")
    # cst rows all identical: [wint(256) | A_cat(255) | C_cat(255) | pad]
    cst = nc.dram_tensor("cst", [128, 768], mybir.dt.float32, kind="ExternalInput")
    outp = nc.dram_tensor("out", [128, TILES], mybir.dt.float32, kind="ExternalOutput")

    lv = _levels()
    A_OFF = 256
    C_OFF = 256 + 255

    with tile.TileContext(nc) as tc:
        with (
            tc.tile_pool(name="const", bufs=1) as constp,
            tc.tile_pool(name="xload", bufs=2) as xp,
            tc.tile_pool(name="leaf", bufs=2) as leafp,
            tc.tile_pool(name="work", bufs=1) as workp,
            tc.tile_pool(name="node", bufs=2) as nodep,
            tc.tile_pool(name="psum", bufs=8, space="PSUM") as psp,
        ):
            wt_sb = constp.tile([128, 2, 256], MM_DT)
            nc.sync.dma_start(out=wt_sb[:, :, :], in_=wt.ap())
            cst_sb = constp.tile([128, 768], mybir.dt.float32)
            nc.sync.dma_start(out=cst_sb[:, :], in_=cst.ap())
            roots = constp.tile([128, TILES], TREE_DT)
            bias_shift = constp.tile([128, 1], mybir.dt.float32)
            nc.vector.memset(bias_shift[:, :], float(BIAS_SHIFT))
            bias_out = constp.tile([128, 1], mybir.dt.float32)
            nc.vector.memset(bias_out[:, :], float(b_out_val))

            def bconst(lo, n):
                """cst slice [128, n] broadcast to [128, T, n]."""
                return (
                    cst_sb[:, lo : lo + n]
                    .rearrange("p (o w) -> p o w", o=1)
                    .broadcast_to([128, T, n])
                )

            eng = {"v": nc.vector, "g": nc.gpsimd}

            for g in range(GROUPS):
                csl = slice(g * GCOLS, (g + 1) * GCOLS)
                xa = xp.tile([128, GCOLS], MM_DT, tag="xa")
                xb = xp.tile([128, GCOLS], MM_DT, tag="xb")
                nc.sync.dma_start(out=xa[:, :], in_=xt.ap()[0, :, csl])
                nc.sync.dma_start(out=xb[:, :], in_=xt.ap()[1, :, csl])

                leafg = leafp.tile([128, T, 256], TREE_DT, tag="leafg")
                for t in range(T):
                    ps = psp.tile([128, 256], mybir.dt.float32, tag="ps")
                    bsl = slice(t * 128, (t + 1) * 128)
                    nc.tensor.matmul(
                        ps[:, :], xa[:, bsl], wt_sb[:, 0, :], start=True, stop=False
                    )
                    nc.tensor.matmul(
                        ps[:, :], xb[:, bsl], wt_sb[:, 1, :], start=False, stop=True
                    )
                    nc.scalar.activation(
                        out=leafg[:, t, :],
                        in_=ps[:, :],
                        func=mybir.ActivationFunctionType.Sigmoid,
                        bias=bias_shift[:, :],
                        scale=float(SHARPNESS),
                    )

                # level-1 child weights: cur = leaf * wint (interleaved)
                cur = workp.tile([128, T, 256], TREE_DT, tag="in1", bufs=2)
                eng[WINT_ENG].tensor_tensor(
                    out=cur[:, :, :],
                    in0=leafg[:, :, :],
                    in1=bconst(0, 256),
                    op=mybir.AluOpType.mult,
                )

                off = 0
                for li, (_, m) in enumerate(lv):
                    p = ENG_PLAN[li]
                    le = cur[:, :, 0::2]
                    ro = cur[:, :, 1::2]
                    s = workp.tile([128, T, m], TREE_DT, tag="s")
                    mx = workp.tile([128, T, m], TREE_DT, tag="mx")
                    q1 = workp.tile([128, T, m], TREE_DT, tag="q1")
                    q2 = workp.tile([128, T, m], TREE_DT, tag="q2")
                    eng[p["s"]].tensor_tensor(
                        out=s[:, :, :], in0=le, in1=ro, op=mybir.AluOpType.add
                    )
                    eng[p["mx"]].tensor_tensor(
                        out=mx[:, :, :], in0=le, in1=ro, op=mybir.AluOpType.max
                    )
                    eng[p["q1"]].scalar_tensor_tensor(
                        out=q1[:, :, :],
                        in0=s[:, :, :],
                        scalar=float(deltas[li]),
                        in1=bconst(A_OFF + off, m),
                        op0=mybir.AluOpType.add,
                        op1=mybir.AluOpType.mult,
                    )
                    eng[p["q2"]].tensor_tensor(
                        out=q2[:, :, :],
                        in0=mx[:, :, :],
                        in1=bconst(C_OFF + off, m),
                        op=mybir.AluOpType.mult,
                    )
                    if m > 1:
                        nxt = nodep.tile([128, T, m], TREE_DT, tag="node")
                        eng[p["nd"]].tensor_tensor(
                            out=nxt[:, :, :],
                            in0=q1[:, :, :],
                            in1=q2[:, :, :],
                            op=mybir.AluOpType.add,
                        )
                        cur = nxt
                    else:
                        rsl = roots[:, g * T : (g + 1) * T].rearrange(
                            "p (t o) -> p t o", o=1
                        )
                        eng[p["nd"]].tensor_tensor(
                            out=rsl,
                            in0=q1[:, :, :],
                            in1=q2[:, :, :],
                            op=mybir.AluOpType.add,
                        )
                    off += m

            final = constp.tile([128, TILES], mybir.dt.float32)
            nc.scalar.activation(
                out=final[:, :],
                in_=roots[:, :],
                func=mybir.ActivationFunctionType.Sigmoid,
                bias=bias_out[:, :],
                scale=1.0,
            )
            nc.sync.dma_start(out=outp.ap(), in_=final[:, :])

    nc.compile()
    return nc


def make_in_maps(x, W_leaf, weights, biases, w_out):
    """Host-side sharding + layout prep. Returns per-core input dicts."""
    np_mm = np.float32  # storage dtype for MM_DT (float32r uses f32 bits)
    wint, A_cat, C_cat, deltas = prep_consts(weights, biases, w_out)

    cst_row = np.zeros(768, np.float32)
    cst_row[0:256] = wint
    cst_row[256 : 256 + 255] = A_cat
    cst_row[256 + 255 : 256 + 510] = C_cat
    cst = np.ascontiguousarray(np.broadcast_to(cst_row, (128, 768)))

    # wt[p, c, l] = W_leaf.T[c*128 + p, l] = W_leaf[l, c*128 + p]
    WT = np.ascontiguousarray(W_leaf.T.astype(np_mm))  # [256, 256] (k, l)
    wt_host = np.ascontiguousarray(WT.reshape(2, 128, 256).transpose(1, 0, 2))

    xT = np.ascontiguousarray(x.T.astype(np_mm))  # [256, B]
    in_maps = []
    for c in range(N_CORES):
        sh = np.ascontiguousarray(
            xT[:, c * BS : (c + 1) * BS].reshape(2, 128, BS)
        )
        in_maps.append({"xt": sh, "wt": wt_host, "cst": cst})
    return in_maps, deltas


def gather_out(results):
    """Per-core [128, TILES] outputs -> full [B, 1]."""
    full = np.empty((B, 1), np.float32)
    for c in range(N_CORES):
        r = np.asarray(results[c]["out"])  # [128, TILES]
        full[c * BS : (c + 1) * BS, 0] = r.T.reshape(BS)
    return full


def kernel(x, W_leaf, weights, biases, w_out, b_out, _run_kwargs=None):
    in_maps, deltas = make_in_maps(x, W_leaf, weights, biases, w_out)
    nc = build_nc(float(b_out[0]), deltas)
    kw = dict(_run_kwargs or {})
    res = run_bass_kernel_spmd(nc, in_maps, core_ids=list(range(N_CORES)), **kw)
    out = gather_out(res.results)
    if _run_kwargs is not None:
        kernel.last_results = res
    return out


# revision 16
# speedup vs baseline: 1.3916x; 1.3916x over previous
"""Trainium2 Bass kernel for nn_BinaryTreeLogicNet.

Computes, for x:[B,256], W_leaf:[256,256], weights:[255,2], biases:[255],
w_out:[1,1], b_out:[1]:

    leaf = sigmoid(x @ W_leaf.T - 2)                       # (B, 256)
    8-level pairwise tree reduce with generalized-gcd nodes # (B, 1)
    out  = sigmoid(root * w_out + b_out)

Key algebraic simplifications (exact up to ~1e-7 absolute):
  - every tree value is positive (sigmoid outputs x positive weights), so
    |.| is a no-op and min/max are plain min/max;
  - lam*min + (1-lam)*max = a*(l+r) + (b-a)*max(l,r) with a = lam, b = 1-lam;
  - each node's output is consumed exactly once, scaled by one weight of the
    next level (w_out at the root), so those weights fold into per-node
    constants A = k*lam, C = k*(1-2lam) computed on the host;
  - the reference's +EPS on both children folds to "+k*EPS after the combine",
    implemented as A*(s + delta) with a per-level immediate delta ~ EPS/lam.

Sharding: pure data parallel over the batch dim across 8 cores. x is
transposed on the host so the contraction dim lands on SBUF partitions and
all DMA is contiguous.
"""

import numpy as np

import concourse.bass as bass
import concourse.bacc as bacc
import concourse.mybir as mybir
import concourse.tile as tile
from concourse.bass_utils import run_bass_kernel_spmd

# ---- problem geometry (hardcoded per contract) ----
B, L = 65536, 256
N_CORES = 8
BS = B // N_CORES            # 8192 rows per core
TILES = BS // 128            # 64 tiles of 128 rows
GROUPS = 4
T = TILES // GROUPS          # 16 tiles per group
GCOLS = T * 128              # 2048 batch columns per group

EPS = 1e-6
SHARPNESS = 1.0
BIAS_SHIFT = -2.0

# dtypes (tunable): matmul path and tree path
MM_DT = mybir.dt.float32r    # fp32 bits, fast PE path (1 cyc/row at N>=256)
TREE_DT = mybir.dt.float16   # 16-bit => DVE 2x mode on contiguous tensor ops
CST_DT = mybir.dt.float16    # tree constants (must match tree dtype for tt)

# engine assignment per tree op: 'v' = vector(DVE), 'g' = gpsimd
# ops: s = l+r, mx = max(l,r), q1 = (s+delta)*A, q2 = mx*C, nd = q1+q2
ENG_PLAN = {
    # level index 0..7 (m = 128 >> li)
    li: {"s": "v", "mx": "v", "q1": "v", "q2": "v", "nd": "v"}
    for li in range(8)
}
WINT_ENG = "v"


def _sigmoid(z):
    return 1.0 / (1.0 + np.exp(-z))


def _levels():
    """[(offset, m)] for m = 128, 64, ..., 1 into the weights/biases arrays."""
    out, off, m = [], 0, 128
    while m >= 1:
        out.append((off, m))
        off += m
        m //= 2
    return out


def _bitrev(n):
    """Bit-reversal permutation of 0..n-1 (involution)."""
    bits = n.bit_length() - 1
    out = np.zeros(n, np.int64)
    for j in range(n):
        r, x = 0, j
        for _ in range(bits):
            r = (r << 1) | (x & 1)
            x >>= 1
        out[j] = r
    return out


def prep_consts(weights, biases, w_out):
    """Host-folded per-node constants, in bit-reversed storage order.

    Level li stores its m output nodes at position q = bitrev(j); with leaves
    stored bit-reversed too, every level's children are the two contiguous
    halves of the previous buffer, elementwise aligned with the outputs.

    Returns (wint[256], A_cat[255], C_cat[255], deltas[8]), A/C concatenated
    level-by-level (sizes 128, 64, ..., 1), each level's slice br-permuted.
    """
    w = weights.astype(np.float64)
    b = biases.astype(np.float64)
    lv = _levels()
    A_parts, C_parts, deltas = [], [], []
    for li, (off, m) in enumerate(lv):
        lam = _sigmoid(b[off : off + m])
        if li + 1 < len(lv):
            noff, nm = lv[li + 1]
            k = np.empty(m, np.float64)
            k[0::2] = w[noff : noff + nm, 0]
            k[1::2] = w[noff : noff + nm, 1]
        else:
            k = np.full(m, float(w_out[0, 0]), np.float64)
        A = lam * k
        C = k * (1.0 - 2.0 * lam)
        br = _bitrev(m)
        A_parts.append(A[br])
        C_parts.append(C[br])
        deltas.append(float(EPS * np.mean(1.0 / lam)))
    wint = np.empty(256, np.float64)
    wint[0::2] = w[0:128, 0]
    wint[1::2] = w[0:128, 1]
    wint = wint[_bitrev(256)]
    return (
        wint.astype(np.float32),
        np.concatenate(A_parts).astype(np.float32),
        np.concatenate(C_parts).astype(np.float32),
        deltas,
    )


def host_emulate(x, W_leaf, weights, biases, w_out, b_out, dtype=np.float32):
    """Pure-numpy emulation of the exact kernel math/layout (for validation)."""
    wint, A_cat, C_cat, deltas = prep_consts(weights, biases, w_out)
    W_perm = W_leaf[_bitrev(256)]  # leaf l lands in column bitrev(l)
    leaf = _sigmoid(
        (x.astype(np.float32) @ W_perm.T.astype(np.float32)) + np.float32(BIAS_SHIFT)
    ).astype(dtype)
    cur = (leaf * wint.astype(dtype)).astype(dtype)
    off = 0
    for li, (_, m) in enumerate(_levels()):
        l_, r_ = cur[:, 0:m], cur[:, m : 2 * m]
        s = (l_ + r_).astype(dtype)
        mx = np.maximum(l_, r_)
        A = A_cat[off : off + m].astype(dtype)
        C = C_cat[off : off + m].astype(dtype)
        cur = (((s + dtype(deltas[li])) * A).astype(dtype) + (mx * C).astype(dtype)).astype(dtype)
        off += m
    return _sigmoid(cur.astype(np.float32) + np.float32(b_out[0]))


def build_nc(b_out_val, deltas):
    """Build the per-core Bass program (SPMD; same NEFF on all cores)."""
    nc = bacc.Bacc("TRN2", target_bir_lowering=False, debug=False)

    xt = nc.dram_tensor("xt", [2, 128, BS], MM_DT, kind="ExternalInput")
    wt = nc.dram_tensor("wt", [128, 2, 256], MM_DT, kind="ExternalInput")
    # cst rows all identical: [wint(256) | A_cat(255) | C_cat(255) | pad]
    cst = nc.dram_tensor("cst", [128, 768], CST_DT, kind="ExternalInput")
    outp = nc.dram_tensor("out", [128, TILES], mybir.dt.float32, kind="ExternalOutput")

    lv = _levels()
    A_OFF = 256
    C_OFF = 256 + 255

    with tile.TileContext(nc) as tc:
        with (
            tc.tile_pool(name="const", bufs=1) as constp,
            tc.tile_pool(name="xload", bufs=2) as xp,
            tc.tile_pool(name="leaf", bufs=2) as leafp,
            tc.tile_pool(name="work", bufs=1) as workp,
            tc.tile_pool(name="node", bufs=2) as nodep,
            tc.tile_pool(name="psum", bufs=8, space="PSUM") as psp,
        ):
            wt_sb = constp.tile([128, 2, 256], MM_DT)
            nc.sync.dma_start(out=wt_sb[:, :, :], in_=wt.ap())
            cst_sb = constp.tile([128, 768], CST_DT)
            nc.sync.dma_start(out=cst_sb[:, :], in_=cst.ap())
            roots = constp.tile([128, TILES], TREE_DT)
            bias_shift = constp.tile([128, 1], mybir.dt.float32)
            nc.vector.memset(bias_shift[:, :], float(BIAS_SHIFT))
            bias_out = constp.tile([128, 1], mybir.dt.float32)
            nc.vector.memset(bias_out[:, :], float(b_out_val))

            def bconst(lo, n):
                """cst slice [128, n] broadcast to [128, T, n]."""
                return (
                    cst_sb[:, lo : lo + n]
                    .rearrange("p (o w) -> p o w", o=1)
                    .broadcast_to([128, T, n])
                )

            eng = {"v": nc.vector, "g": nc.gpsimd}

            for g in range(GROUPS):
                csl = slice(g * GCOLS, (g + 1) * GCOLS)
                xa = xp.tile([128, GCOLS], MM_DT, tag="xa")
                xb = xp.tile([128, GCOLS], MM_DT, tag="xb")
                nc.sync.dma_start(out=xa[:, :], in_=xt.ap()[0, :, csl])
                nc.sync.dma_start(out=xb[:, :], in_=xt.ap()[1, :, csl])

                leafg = leafp.tile([128, T, 256], TREE_DT, tag="leafg")
                for t in range(T):
                    ps = psp.tile([128, 256], mybir.dt.float32, tag="ps")
                    bsl = slice(t * 128, (t + 1) * 128)
                    nc.tensor.matmul(
                        ps[:, :], xa[:, bsl], wt_sb[:, 0, :], start=True, stop=False
                    )
                    nc.tensor.matmul(
                        ps[:, :], xb[:, bsl], wt_sb[:, 1, :], start=False, stop=True
                    )
                    nc.scalar.activation(
                        out=leafg[:, t, :],
                        in_=ps[:, :],
                        func=mybir.ActivationFunctionType.Sigmoid,
                        bias=bias_shift[:, :],
                        scale=float(SHARPNESS),
                    )

                # level-1 child weights: cur = leaf * wint (interleaved)
                cur = workp.tile([128, T, 256], TREE_DT, tag="in1", bufs=2)
                eng[WINT_ENG].tensor_tensor(
                    out=cur[:, :, :],
                    in0=leafg[:, :, :],
                    in1=bconst(0, 256),
                    op=mybir.AluOpType.mult,
                )

                off = 0
                for li, (_, m) in enumerate(lv):
                    p = ENG_PLAN[li]
                    le = cur[:, :, 0:m]
                    ro = cur[:, :, m : 2 * m]
                    s = workp.tile([128, T, m], TREE_DT, tag="s")
                    mx = workp.tile([128, T, m], TREE_DT, tag="mx")
                    q1 = workp.tile([128, T, m], TREE_DT, tag="q1")
                    q2 = workp.tile([128, T, m], TREE_DT, tag="q2")
                    eng[p["s"]].tensor_tensor(
                        out=s[:, :, :], in0=le, in1=ro, op=mybir.AluOpType.add
                    )
                    eng[p["mx"]].tensor_tensor(
                        out=mx[:, :, :], in0=le, in1=ro, op=mybir.AluOpType.max
                    )
                    eng[p["q1"]].scalar_tensor_tensor(
                        out=q1[:, :, :],
                        in0=s[:, :, :],
                        scalar=float(deltas[li]),
                        in1=bconst(A_OFF + off, m),
                        op0=mybir.AluOpType.add,
                        op1=mybir.AluOpType.mult,
                    )
                    eng[p["q2"]].tensor_tensor(
                        out=q2[:, :, :],
                        in0=mx[:, :, :],
                        in1=bconst(C_OFF + off, m),
                        op=mybir.AluOpType.mult,
                    )
                    if m > 1:
                        nxt = nodep.tile([128, T, m], TREE_DT, tag="node")
                        eng[p["nd"]].tensor_tensor(
                            out=nxt[:, :, :],
                            in0=q1[:, :, :],
                            in1=q2[:, :, :],
                            op=mybir.AluOpType.add,
                        )
                        cur = nxt
                    else:
                        rsl = roots[:, g * T : (g + 1) * T].rearrange(
                            "p (t o) -> p t o", o=1
                        )
                        eng[p["nd"]].tensor_tensor(
                            out=rsl,
                            in0=q1[:, :, :],
                            in1=q2[:, :, :],
                            op=mybir.AluOpType.add,
                        )
                    off += m

            final = constp.tile([128, TILES], mybir.dt.float32)
            nc.scalar.activation(
                out=final[:, :],
                in_=roots[:, :],
                func=mybir.ActivationFunctionType.Sigmoid,
                bias=bias_out[:, :],
                scale=1.0,
            )
            nc.sync.dma_start(out=outp.ap(), in_=final[:, :])

    nc.compile()
    return nc


def make_in_maps(x, W_leaf, weights, biases, w_out):
    """Host-side sharding + layout prep. Returns per-core input dicts."""
    np_mm = np.float32  # storage dtype for MM_DT (float32r uses f32 bits)
    np_cst = np.float16
    wint, A_cat, C_cat, deltas = prep_consts(weights, biases, w_out)

    cst_row = np.zeros(768, np_cst)
    cst_row[0:256] = wint.astype(np_cst)
    cst_row[256 : 256 + 255] = A_cat.astype(np_cst)
    cst_row[256 + 255 : 256 + 510] = C_cat.astype(np_cst)
    cst = np.ascontiguousarray(np.broadcast_to(cst_row, (128, 768)))

    # leaf l lands in column bitrev(l); wt[p, c, l] = W_perm[l, c*128 + p]
    W_perm = W_leaf[_bitrev(256)]
    WT = np.ascontiguousarray(W_perm.T.astype(np_mm))  # [256, 256] (k, l)
    wt_host = np.ascontiguousarray(WT.reshape(2, 128, 256).transpose(1, 0, 2))

    xT = np.ascontiguousarray(x.T.astype(np_mm))  # [256, B]
    in_maps = []
    for c in range(N_CORES):
        sh = np.ascontiguousarray(
            xT[:, c * BS : (c + 1) * BS].reshape(2, 128, BS)
        )
        in_maps.append({"xt": sh, "wt": wt_host, "cst": cst})
    return in_maps, deltas


def gather_out(results):
    """Per-core [128, TILES] outputs -> full [B, 1]."""
    full = np.empty((B, 1), np.float32)
    for c in range(N_CORES):
        r = np.asarray(results[c]["out"])  # [128, TILES]
        full[c * BS : (c + 1) * BS, 0] = r.T.reshape(BS)
    return full


def kernel(x, W_leaf, weights, biases, w_out, b_out, _run_kwargs=None):
    in_maps, deltas = make_in_maps(x, W_leaf, weights, biases, w_out)
    nc = build_nc(float(b_out[0]), deltas)
    kw = dict(_run_kwargs or {})
    res = run_bass_kernel_spmd(nc, in_maps, core_ids=list(range(N_CORES)), **kw)
    out = gather_out(res.results)
    if _run_kwargs is not None:
        kernel.last_results = res
    return out


# revision 20
# speedup vs baseline: 1.4601x; 1.0493x over previous
"""Trainium2 Bass kernel for nn_BinaryTreeLogicNet.

Computes, for x:[B,256], W_leaf:[256,256], weights:[255,2], biases:[255],
w_out:[1,1], b_out:[1]:

    leaf = sigmoid(x @ W_leaf.T - 2)                       # (B, 256)
    8-level pairwise tree reduce with generalized-gcd nodes # (B, 1)
    out  = sigmoid(root * w_out + b_out)

Key algebraic simplifications (exact up to ~1e-7 absolute):
  - every tree value is positive (sigmoid outputs x positive weights), so
    |.| is a no-op and min/max are plain min/max;
  - lam*min + (1-lam)*max = a*(l+r) + (b-a)*max(l,r) with a = lam, b = 1-lam;
  - each node's output is consumed exactly once, scaled by one weight of the
    next level (w_out at the root), so those weights fold into per-node
    constants A = k*lam, C = k*(1-2lam) computed on the host;
  - the reference's +EPS on both children folds to "+k*EPS after the combine",
    implemented as A*(s + delta) with a per-level immediate delta ~ EPS/lam.

Sharding: pure data parallel over the batch dim across 8 cores. x is
transposed on the host so the contraction dim lands on SBUF partitions and
all DMA is contiguous.
"""

import numpy as np

import concourse.bass as bass
import concourse.bacc as bacc
import concourse.mybir as mybir
import concourse.tile as tile
from concourse.bass_utils import run_bass_kernel_spmd

# ---- problem geometry (hardcoded per contract) ----
B, L = 65536, 256
N_CORES = 8
BS = B // N_CORES            # 8192 rows per core
TILES = BS // 128            # 64 tiles of 128 rows
GROUPS = 4
T = TILES // GROUPS          # 16 tiles per group
GCOLS = T * 128              # 2048 batch columns per group

EPS = 1e-6
SHARPNESS = 1.0
BIAS_SHIFT = -2.0

# dtypes (tunable): matmul path and tree path
MM_DT = mybir.dt.float32r    # fp32 bits, fast PE path (1 cyc/row at N>=256)
TREE_DT = mybir.dt.float16   # 16-bit => DVE 2x mode on contiguous tensor ops
CST_DT = mybir.dt.float16    # tree constants (must match tree dtype for tt)

# engine assignment per tree op: 'v' = vector(DVE), 'g' = gpsimd
# ops: s = l+r, mx = max(l,r), q1 = (s+delta)*A, q2 = mx*C, nd = q1+q2
ENG_PLAN = {
    # level index 0..7 (m = 128 >> li)
    li: {"s": "v", "mx": "v", "q1": "v", "q2": "v", "nd": "v"}
    for li in range(8)
}
WINT_ENG = "v"


def _sigmoid(z):
    return 1.0 / (1.0 + np.exp(-z))


def _levels():
    """[(offset, m)] for m = 128, 64, ..., 1 into the weights/biases arrays."""
    out, off, m = [], 0, 128
    while m >= 1:
        out.append((off, m))
        off += m
        m //= 2
    return out


def _bitrev(n):
    """Bit-reversal permutation of 0..n-1 (involution)."""
    bits = n.bit_length() - 1
    out = np.zeros(n, np.int64)
    for j in range(n):
        r, x = 0, j
        for _ in range(bits):
            r = (r << 1) | (x & 1)
            x >>= 1
        out[j] = r
    return out


def prep_consts(weights, biases, w_out):
    """Host-folded per-node constants, in bit-reversed storage order.

    Level li stores its m output nodes at position q = bitrev(j); with leaves
    stored bit-reversed too, every level's children are the two contiguous
    halves of the previous buffer, elementwise aligned with the outputs.

    Returns (wint[256], A_cat[255], C_cat[255], deltas[8]), A/C concatenated
    level-by-level (sizes 128, 64, ..., 1), each level's slice br-permuted.
    """
    w = weights.astype(np.float64)
    b = biases.astype(np.float64)
    lv = _levels()
    A_parts, C_parts, deltas = [], [], []
    for li, (off, m) in enumerate(lv):
        lam = _sigmoid(b[off : off + m])
        if li + 1 < len(lv):
            noff, nm = lv[li + 1]
            k = np.empty(m, np.float64)
            k[0::2] = w[noff : noff + nm, 0]
            k[1::2] = w[noff : noff + nm, 1]
        else:
            k = np.full(m, float(w_out[0, 0]), np.float64)
        A = lam * k
        C = k * (1.0 - 2.0 * lam)
        br = _bitrev(m)
        A_parts.append(A[br])
        C_parts.append(C[br])
        # EPS's exact contribution is k*EPS per node (~1e-6 absolute); with a
        # 16-bit tree it is far below storage rounding, so it is dropped in
        # the kernel. deltas kept for the fp32 emulation path only.
        deltas.append(float(EPS * np.mean(1.0 / lam)))
    wint = np.empty(256, np.float64)
    wint[0::2] = w[0:128, 0]
    wint[1::2] = w[0:128, 1]
    wint = wint[_bitrev(256)]
    return (
        wint.astype(np.float32),
        np.concatenate(A_parts).astype(np.float32),
        np.concatenate(C_parts).astype(np.float32),
        deltas,
    )


def host_emulate(x, W_leaf, weights, biases, w_out, b_out, dtype=np.float32):
    """Pure-numpy emulation of the exact kernel math/layout (for validation)."""
    wint, A_cat, C_cat, deltas = prep_consts(weights, biases, w_out)
    W_perm = W_leaf[_bitrev(256)]  # leaf l lands in column bitrev(l)
    leaf = _sigmoid(
        (x.astype(np.float32) @ W_perm.T.astype(np.float32)) + np.float32(BIAS_SHIFT)
    ).astype(dtype)
    cur = (leaf * wint.astype(dtype)).astype(dtype)
    off = 0
    for li, (_, m) in enumerate(_levels()):
        l_, r_ = cur[:, 0:m], cur[:, m : 2 * m]
        s = (l_ + r_).astype(dtype)
        mx = np.maximum(l_, r_)
        A = A_cat[off : off + m].astype(dtype)
        C = C_cat[off : off + m].astype(dtype)
        cur = ((s * A).astype(dtype) + (mx * C).astype(dtype)).astype(dtype)
        off += m
    return _sigmoid(cur.astype(np.float32) + np.float32(b_out[0]))


def build_nc(b_out_val, deltas):
    """Build the per-core Bass program (SPMD; same NEFF on all cores)."""
    nc = bacc.Bacc("TRN2", target_bir_lowering=False, debug=False)

    xt = nc.dram_tensor("xt", [2, 128, BS], MM_DT, kind="ExternalInput")
    wt = nc.dram_tensor("wt", [128, 2, 256], MM_DT, kind="ExternalInput")
    # cst rows all identical: [wint(256) | A_cat(255) | C_cat(255) | pad]
    cst = nc.dram_tensor("cst", [128, 768], CST_DT, kind="ExternalInput")
    outp = nc.dram_tensor("out", [128, TILES], mybir.dt.float32, kind="ExternalOutput")

    lv = _levels()
    A_OFF = 256
    C_OFF = 256 + 255

    with tile.TileContext(nc) as tc:
        with (
            tc.tile_pool(name="const", bufs=1) as constp,
            tc.tile_pool(name="xload", bufs=2) as xp,
            tc.tile_pool(name="leaf", bufs=2) as leafp,
            tc.tile_pool(name="work", bufs=1) as workp,
            tc.tile_pool(name="node", bufs=2) as nodep,
            tc.tile_pool(name="psum", bufs=8, space="PSUM") as psp,
        ):
            wt_sb = constp.tile([128, 2, 256], MM_DT)
            nc.sync.dma_start(out=wt_sb[:, :, :], in_=wt.ap())
            cst_sb = constp.tile([128, 768], CST_DT)
            nc.sync.dma_start(out=cst_sb[:, :], in_=cst.ap())
            roots = constp.tile([128, TILES], TREE_DT)
            bias_shift = constp.tile([128, 1], mybir.dt.float32)
            nc.vector.memset(bias_shift[:, :], float(BIAS_SHIFT))
            bias_out = constp.tile([128, 1], mybir.dt.float32)
            nc.vector.memset(bias_out[:, :], float(b_out_val))

            def bconst(lo, n):
                """cst slice [128, n] broadcast to [128, T, n]."""
                return (
                    cst_sb[:, lo : lo + n]
                    .rearrange("p (o w) -> p o w", o=1)
                    .broadcast_to([128, T, n])
                )

            eng = {"v": nc.vector, "g": nc.gpsimd}

            for g in range(GROUPS):
                csl = slice(g * GCOLS, (g + 1) * GCOLS)
                xa = xp.tile([128, GCOLS], MM_DT, tag="xa")
                xb = xp.tile([128, GCOLS], MM_DT, tag="xb")
                nc.sync.dma_start(out=xa[:, :], in_=xt.ap()[0, :, csl])
                nc.sync.dma_start(out=xb[:, :], in_=xt.ap()[1, :, csl])

                leafg = leafp.tile([128, T, 256], TREE_DT, tag="leafg")
                for tp in range(T // 2):
                    ps = psp.tile([128, 2, 256], mybir.dt.float32, tag="ps")
                    for half in range(2):
                        t = 2 * tp + half
                        bsl = slice(t * 128, (t + 1) * 128)
                        nc.tensor.matmul(
                            ps[:, half, :],
                            xa[:, bsl],
                            wt_sb[:, 0, :],
                            start=True,
                            stop=False,
                        )
                        nc.tensor.matmul(
                            ps[:, half, :],
                            xb[:, bsl],
                            wt_sb[:, 1, :],
                            start=False,
                            stop=True,
                        )
                    nc.scalar.activation(
                        out=leafg[:, 2 * tp : 2 * tp + 2, :],
                        in_=ps[:, :, :],
                        func=mybir.ActivationFunctionType.Sigmoid,
                        bias=bias_shift[:, :],
                        scale=float(SHARPNESS),
                    )

                # level-1 child weights: cur = leaf * wint (interleaved)
                cur = workp.tile([128, T, 256], TREE_DT, tag="in1", bufs=2)
                eng[WINT_ENG].tensor_tensor(
                    out=cur[:, :, :],
                    in0=leafg[:, :, :],
                    in1=bconst(0, 256),
                    op=mybir.AluOpType.mult,
                )

                off = 0
                for li, (_, m) in enumerate(lv):
                    p = ENG_PLAN[li]
                    le = cur[:, :, 0:m]
                    ro = cur[:, :, m : 2 * m]
                    s = workp.tile([128, T, m], TREE_DT, tag="s")
                    mx = workp.tile([128, T, m], TREE_DT, tag="mx")
                    q1 = workp.tile([128, T, m], TREE_DT, tag="q1")
                    q2 = workp.tile([128, T, m], TREE_DT, tag="q2")
                    eng[p["s"]].tensor_tensor(
                        out=s[:, :, :], in0=le, in1=ro, op=mybir.AluOpType.add
                    )
                    eng[p["mx"]].tensor_tensor(
                        out=mx[:, :, :], in0=le, in1=ro, op=mybir.AluOpType.max
                    )
                    eng[p["q1"]].tensor_tensor(
                        out=q1[:, :, :],
                        in0=s[:, :, :],
                        in1=bconst(A_OFF + off, m),
                        op=mybir.AluOpType.mult,
                    )
                    eng[p["q2"]].tensor_tensor(
                        out=q2[:, :, :],
                        in0=mx[:, :, :],
                        in1=bconst(C_OFF + off, m),
                        op=mybir.AluOpType.mult,
                    )
                    if m > 1:
                        nxt = nodep.tile([128, T, m], TREE_DT, tag="node")
                        eng[p["nd"]].tensor_tensor(
                            out=nxt[:, :, :],
                            in0=q1[:, :, :],
                            in1=q2[:, :, :],
                            op=mybir.AluOpType.add,
                        )
                        cur = nxt
                    else:
                        rsl = roots[:, g * T : (g + 1) * T].rearrange(
                            "p (t o) -> p t o", o=1
                        )
                        eng[p["nd"]].tensor_tensor(
                            out=rsl,
                            in0=q1[:, :, :],
                            in1=q2[:, :, :],
                            op=mybir.AluOpType.add,
                        )
                    off += m

            final = constp.tile([128, TILES], mybir.dt.float32)
            nc.scalar.activation(
                out=final[:, :],
                in_=roots[:, :],
                func=mybir.ActivationFunctionType.Sigmoid,
                bias=bias_out[:, :],
                scale=1.0,
            )
            nc.sync.dma_start(out=outp.ap(), in_=final[:, :])

    nc.compile()
    return nc


def make_in_maps(x, W_leaf, weights, biases, w_out):
    """Host-side sharding + layout prep. Returns per-core input dicts."""
    np_mm = np.float32  # storage dtype for MM_DT (float32r uses f32 bits)
    np_cst = np.float16
    wint, A_cat, C_cat, deltas = prep_consts(weights, biases, w_out)

    cst_row = np.zeros(768, np_cst)
    cst_row[0:256] = wint.astype(np_cst)
    cst_row[256 : 256 + 255] = A_cat.astype(np_cst)
    cst_row[256 + 255 : 256 + 510] = C_cat.astype(np_cst)
    cst = np.ascontiguousarray(np.broadcast_to(cst_row, (128, 768)))

    # leaf l lands in column bitrev(l); wt[p, c, l] = W_perm[l, c*128 + p]
    W_perm = W_leaf[_bitrev(256)]
    WT = np.ascontiguousarray(W_perm.T.astype(np_mm))  # [256, 256] (k, l)
    wt_host = np.ascontiguousarray(WT.reshape(2, 128, 256).transpose(1, 0, 2))

    xT = np.ascontiguousarray(x.T.astype(np_mm))  # [256, B]
    in_maps = []
    for c in range(N_CORES):
        sh = np.ascontiguousarray(
            xT[:, c * BS : (c + 1) * BS].reshape(2, 128, BS)
        )
        in_maps.append({"xt": sh, "wt": wt_host, "cst": cst})
    return in_maps, deltas


def gather_out(results):
    """Per-core [128, TILES] outputs -> full [B, 1]."""
    full = np.empty((B, 1), np.float32)
    for c in range(N_CORES):
        r = np.asarray(results[c]["out"])  # [128, TILES]
        full[c * BS : (c + 1) * BS, 0] = r.T.reshape(BS)
    return full


def kernel(x, W_leaf, weights, biases, w_out, b_out, _run_kwargs=None):
    in_maps, deltas = make_in_maps(x, W_leaf, weights, biases, w_out)
    nc = build_nc(float(b_out[0]), deltas)
    kw = dict(_run_kwargs or {})
    res = run_bass_kernel_spmd(nc, in_maps, core_ids=list(range(N_CORES)), **kw)
    out = gather_out(res.results)
    if _run_kwargs is not None:
        kernel.last_results = res
    return out


# revision 28
# speedup vs baseline: 1.8122x; 1.2411x over previous
"""Trainium2 Bass kernel for nn_BinaryTreeLogicNet.

Computes, for x:[B,256], W_leaf:[256,256], weights:[255,2], biases:[255],
w_out:[1,1], b_out:[1]:

    leaf = sigmoid(x @ W_leaf.T - 2)                       # (B, 256)
    8-level pairwise tree reduce with generalized-gcd nodes # (B, 1)
    out  = sigmoid(root * w_out + b_out)

Key algebraic simplifications (exact up to ~1e-7 absolute):
  - every tree value is positive (sigmoid outputs x positive weights), so
    |.| is a no-op and min/max are plain min/max;
  - lam*min + (1-lam)*max = a*(l+r) + (b-a)*max(l,r) with a = lam, b = 1-lam;
  - each node's output is consumed exactly once, scaled by one weight of the
    next level (w_out at the root), so those weights fold into per-node
    constants A = k*lam, C = k*(1-2lam) computed on the host;
  - the reference's +EPS on both children folds to "+k*EPS after the combine",
    implemented as A*(s + delta) with a per-level immediate delta ~ EPS/lam.

Sharding: pure data parallel over the batch dim across 8 cores. x is
transposed on the host so the contraction dim lands on SBUF partitions and
all DMA is contiguous.
"""

import numpy as np

import concourse.bass as bass
import concourse.bacc as bacc
import concourse.mybir as mybir
import concourse.tile as tile
from concourse.bass_utils import run_bass_kernel_spmd

# ---- problem geometry (hardcoded per contract) ----
B, L = 65536, 256
N_CORES = 8
BS = B // N_CORES            # 8192 rows per core
TILES = BS // 128            # 64 tiles of 128 rows
GROUPS = 2
T = TILES // GROUPS          # 32 tiles per tree group
XSUB = 8                     # x tiles per DMA chunk (pipelining granularity)
RHO = 128.0                  # global pow2 rescale for the A-folded tree

EPS = 1e-6
SHARPNESS = 1.0
BIAS_SHIFT = -2.0

# dtypes (tunable): matmul path and tree path
MM_DT = mybir.dt.float32r    # fp32 bits, fast PE path (1 cyc/row at N>=256)
TREE_DT = mybir.dt.float16   # 16-bit => DVE 2x mode on contiguous tensor ops
CST_DT = mybir.dt.float16    # tree constants (must match tree dtype for tt)

# engine assignment per tree op: 'v' = vector(DVE), 'g' = gpsimd
# ops: s = l+r, mx = max(l,r), q1 = (s+delta)*A, q2 = mx*C, nd = q1+q2
ENG_PLAN = {
    # level index 0..7 (m = 128 >> li)
    li: {"s": "v", "mx": "v", "q1": "v", "q2": "v", "nd": "v"}
    for li in range(8)
}
WINT_ENG = "v"


def _sigmoid(z):
    return 1.0 / (1.0 + np.exp(-z))


def _levels():
    """[(offset, m)] for m = 128, 64, ..., 1 into the weights/biases arrays."""
    out, off, m = [], 0, 128
    while m >= 1:
        out.append((off, m))
        off += m
        m //= 2
    return out


def _bitrev(n):
    """Bit-reversal permutation of 0..n-1 (involution)."""
    bits = n.bit_length() - 1
    out = np.zeros(n, np.int64)
    for j in range(n):
        r, x = 0, j
        for _ in range(bits):
            r = (r << 1) | (x & 1)
            x >>= 1
        out[j] = r
    return out


def prep_consts(weights, biases, w_out):
    """Host-folded per-node constants, A-folded, in bit-reversed order.

    Each node's A = lam*k coefficient is pushed down into its children's
    scales (sigma chain, anchored at sigma_root = RHO for fp16 range), so
    levels 0..6 need only  node = (l + r) + Chat*max(l, r)  with
    Chat = C/A.  The root level keeps explicit A'=A/RHO, C'=C/RHO.

    Level li stores its m output nodes at position q = bitrev(j); with leaves
    stored bit-reversed too, every level's children are the two contiguous
    halves of the previous buffer, elementwise aligned with the outputs.

    Returns (wint[256], Chat_cat[254], a7, c7) in float64 (orig math),
    br-permuted, ready to cast.
    """
    w = weights.astype(np.float64)
    b = biases.astype(np.float64)
    lv = _levels()
    A_lv, C_lv = [], []
    for li, (off, m) in enumerate(lv):
        lam = _sigmoid(b[off : off + m])
        if li + 1 < len(lv):
            noff, nm = lv[li + 1]
            k = np.empty(m, np.float64)
            k[0::2] = w[noff : noff + nm, 0]
            k[1::2] = w[noff : noff + nm, 1]
        else:
            k = np.full(m, float(w_out[0, 0]), np.float64)
        A_lv.append(lam * k)
        C_lv.append(k * (1.0 - 2.0 * lam))
    # sigma chain: sig[li][j] = scale of level-li node j's stored value.
    # Levels 0..6 use the A-folded 4-op form, so each level's A goes into its
    # children's sigma; the root (level 7) keeps its A explicit, so its
    # children carry only the RHO range-rescale.
    sig = [None] * 7
    sig[6] = np.full(2, RHO)
    for li in range(5, -1, -1):
        j = np.arange(128 >> li)
        sig[li] = sig[li + 1][j >> 1] * A_lv[li + 1][j >> 1]
    l_idx = np.arange(256)
    leaf_sig = sig[0][l_idx >> 1] * A_lv[0][l_idx >> 1]

    wint = np.empty(256, np.float64)
    wint[0::2] = w[0:128, 0]
    wint[1::2] = w[0:128, 1]
    wint = (wint * leaf_sig)[_bitrev(256)]

    Chat_parts = [
        (C_lv[li] / A_lv[li])[_bitrev(128 >> li)] for li in range(7)
    ]
    a7 = float(A_lv[7][0] / RHO)
    c7 = float(C_lv[7][0] / RHO)
    return wint, np.concatenate(Chat_parts), a7, c7


def host_emulate(x, W_leaf, weights, biases, w_out, b_out, dtype=np.float32):
    """Pure-numpy emulation of the exact kernel math/layout (for validation)."""
    wint, Chat_cat, a7, c7 = prep_consts(weights, biases, w_out)
    W_perm = W_leaf[_bitrev(256)]  # leaf l lands in column bitrev(l)
    leaf = _sigmoid(
        (x.astype(np.float32) @ W_perm.T.astype(np.float32)) + np.float32(BIAS_SHIFT)
    ).astype(dtype)
    cur = (leaf * wint.astype(dtype)).astype(dtype)
    off = 0
    for li in range(7):
        m = 128 >> li
        l_, r_ = cur[:, 0:m], cur[:, m : 2 * m]
        s = (l_ + r_).astype(dtype)
        mx = np.maximum(l_, r_)
        Ch = Chat_cat[off : off + m].astype(dtype)
        cur = (s + (mx * Ch).astype(dtype)).astype(dtype)
        off += m
    l_, r_ = cur[:, 0:1], cur[:, 1:2]
    s = (l_ + r_).astype(dtype)
    mx = np.maximum(l_, r_)
    cur = ((s * dtype(a7)).astype(dtype) + (mx * dtype(c7)).astype(dtype)).astype(dtype)
    return _sigmoid(cur.astype(np.float32) + np.float32(b_out[0]))


def build_nc(b_out_val, a7, c7):
    """Build the per-core Bass program (SPMD; same NEFF on all cores)."""
    nc = bacc.Bacc("TRN2", target_bir_lowering=False, debug=False)

    xt = nc.dram_tensor("xt", [2, 128, BS], MM_DT, kind="ExternalInput")
    wt = nc.dram_tensor("wt", [128, 2, 256], MM_DT, kind="ExternalInput")
    # cst rows all identical: [wint'(256) | Chat_cat(254) | pad]
    cst = nc.dram_tensor("cst", [128, 512], CST_DT, kind="ExternalInput")
    outp = nc.dram_tensor("out", [128, TILES], mybir.dt.float32, kind="ExternalOutput")

    CHAT_OFF = 256
    XCOLS = XSUB * 128

    with tile.TileContext(nc) as tc:
        with (
            tc.tile_pool(name="const", bufs=1) as constp,
            tc.tile_pool(name="xload", bufs=3) as xp,
            tc.tile_pool(name="leaf", bufs=2) as leafp,
            tc.tile_pool(name="work", bufs=1) as workp,
            tc.tile_pool(name="node", bufs=2) as nodep,
            tc.tile_pool(name="psum", bufs=8, space="PSUM") as psp,
        ):
            wt_sb = constp.tile([128, 2, 256], MM_DT)
            nc.sync.dma_start(out=wt_sb[:, :, :], in_=wt.ap())
            cst_sb = constp.tile([128, 512], CST_DT)
            nc.sync.dma_start(out=cst_sb[:, :], in_=cst.ap())
            roots = constp.tile([128, TILES], TREE_DT)
            bias_shift = constp.tile([128, 1], mybir.dt.float32)
            nc.vector.memset(bias_shift[:, :], float(BIAS_SHIFT))
            bias_out = constp.tile([128, 1], mybir.dt.float32)
            nc.vector.memset(bias_out[:, :], float(b_out_val))

            def bconst(lo, n):
                """cst slice [128, n] broadcast to [128, T, n]."""
                return (
                    cst_sb[:, lo : lo + n]
                    .rearrange("p (o w) -> p o w", o=1)
                    .broadcast_to([128, T, n])
                )

            eng = {"v": nc.vector, "g": nc.gpsimd}

            for g in range(GROUPS):
                leafg = leafp.tile([128, T, 256], TREE_DT, tag="leafg")
                for sub in range(T // XSUB):
                    xoff = g * T * 128 + sub * XCOLS
                    xa = xp.tile([128, XCOLS], MM_DT, tag="xa")
                    xb = xp.tile([128, XCOLS], MM_DT, tag="xb")
                    nc.sync.dma_start(
                        out=xa[:, :], in_=xt.ap()[0, :, xoff : xoff + XCOLS]
                    )
                    nc.sync.dma_start(
                        out=xb[:, :], in_=xt.ap()[1, :, xoff : xoff + XCOLS]
                    )
                    for tp in range(XSUB // 2):
                        ps = psp.tile([128, 2, 256], mybir.dt.float32, tag="ps")
                        for half in range(2):
                            tl = 2 * tp + half  # tile within sub-block
                            bsl = slice(tl * 128, (tl + 1) * 128)
                            nc.tensor.matmul(
                                ps[:, half, :],
                                xa[:, bsl],
                                wt_sb[:, 0, :],
                                start=True,
                                stop=False,
                            )
                            nc.tensor.matmul(
                                ps[:, half, :],
                                xb[:, bsl],
                                wt_sb[:, 1, :],
                                start=False,
                                stop=True,
                            )
                        t0 = sub * XSUB + 2 * tp
                        nc.scalar.activation(
                            out=leafg[:, t0 : t0 + 2, :],
                            in_=ps[:, :, :],
                            func=mybir.ActivationFunctionType.Sigmoid,
                            bias=bias_shift[:, :],
                            scale=float(SHARPNESS),
                        )

                # level-0 child weights (sigma-folded): cur = leaf * wint'
                cur = workp.tile([128, T, 256], TREE_DT, tag="in1", bufs=2)
                eng[WINT_ENG].tensor_tensor(
                    out=cur[:, :, :],
                    in0=leafg[:, :, :],
                    in1=bconst(0, 256),
                    op=mybir.AluOpType.mult,
                )

                off = 0
                for li in range(7):
                    m = 128 >> li
                    p = ENG_PLAN[li]
                    le = cur[:, :, 0:m]
                    ro = cur[:, :, m : 2 * m]
                    s = workp.tile([128, T, m], TREE_DT, tag="s")
                    mx = workp.tile([128, T, m], TREE_DT, tag="mx")
                    q2 = workp.tile([128, T, m], TREE_DT, tag="q2")
                    nxt = nodep.tile([128, T, m], TREE_DT, tag="node")
                    eng[p["s"]].tensor_tensor(
                        out=s[:, :, :], in0=le, in1=ro, op=mybir.AluOpType.add
                    )
                    eng[p["mx"]].tensor_tensor(
                        out=mx[:, :, :], in0=le, in1=ro, op=mybir.AluOpType.max
                    )
                    eng[p["q2"]].tensor_tensor(
                        out=q2[:, :, :],
                        in0=mx[:, :, :],
                        in1=bconst(CHAT_OFF + off, m),
                        op=mybir.AluOpType.mult,
                    )
                    eng[p["nd"]].tensor_tensor(
                        out=nxt[:, :, :],
                        in0=s[:, :, :],
                        in1=q2[:, :, :],
                        op=mybir.AluOpType.add,
                    )
                    cur = nxt
                    off += m

                # root level: explicit A' = A/RHO, C' = C/RHO immediates
                s = workp.tile([128, T, 1], TREE_DT, tag="s7")
                mx = workp.tile([128, T, 1], TREE_DT, tag="mx7")
                q2 = workp.tile([128, T, 1], TREE_DT, tag="q27")
                nc.vector.tensor_tensor(
                    out=s[:, :, :],
                    in0=cur[:, :, 0:1],
                    in1=cur[:, :, 1:2],
                    op=mybir.AluOpType.add,
                )
                nc.vector.tensor_tensor(
                    out=mx[:, :, :],
                    in0=cur[:, :, 0:1],
                    in1=cur[:, :, 1:2],
                    op=mybir.AluOpType.max,
                )
                nc.vector.tensor_scalar_mul(out=q2[:, :, :], in0=mx[:, :, :], scalar1=float(c7))
                rsl = roots[:, g * T : (g + 1) * T].rearrange("p (t o) -> p t o", o=1)
                nc.vector.scalar_tensor_tensor(
                    out=rsl,
                    in0=s[:, :, :],
                    scalar=float(a7),
                    in1=q2[:, :, :],
                    op0=mybir.AluOpType.mult,
                    op1=mybir.AluOpType.add,
                )

            final = constp.tile([128, TILES], mybir.dt.float32)
            nc.scalar.activation(
                out=final[:, :],
                in_=roots[:, :],
                func=mybir.ActivationFunctionType.Sigmoid,
                bias=bias_out[:, :],
                scale=1.0,
            )
            nc.sync.dma_start(out=outp.ap(), in_=final[:, :])

    nc.compile()
    return nc


def make_in_maps(x, W_leaf, weights, biases, w_out):
    """Host-side sharding + layout prep. Returns per-core input dicts."""
    np_mm = np.float32  # storage dtype for MM_DT (float32r uses f32 bits)
    np_cst = np.float16
    wint, Chat_cat, a7, c7 = prep_consts(weights, biases, w_out)

    cst_row = np.zeros(512, np_cst)
    cst_row[0:256] = wint.astype(np_cst)
    cst_row[256 : 256 + 254] = Chat_cat.astype(np_cst)
    cst = np.ascontiguousarray(np.broadcast_to(cst_row, (128, 512)))

    # leaf l lands in column bitrev(l); wt[p, c, l] = W_perm[l, c*128 + p]
    W_perm = W_leaf[_bitrev(256)]
    WT = np.ascontiguousarray(W_perm.T.astype(np_mm))  # [256, 256] (k, l)
    wt_host = np.ascontiguousarray(WT.reshape(2, 128, 256).transpose(1, 0, 2))

    xT = np.ascontiguousarray(x.T.astype(np_mm))  # [256, B]
    in_maps = []
    for c in range(N_CORES):
        sh = np.ascontiguousarray(
            xT[:, c * BS : (c + 1) * BS].reshape(2, 128, BS)
        )
        in_maps.append({"xt": sh, "wt": wt_host, "cst": cst})
    return in_maps, a7, c7


def gather_out(results):
    """Per-core [128, TILES] outputs -> full [B, 1]."""
    full = np.empty((B, 1), np.float32)
    for c in range(N_CORES):
        r = np.asarray(results[c]["out"])  # [128, TILES]
        full[c * BS : (c + 1) * BS, 0] = r.T.reshape(BS)
    return full


def kernel(x, W_leaf, weights, biases, w_out, b_out, _run_kwargs=None):
    in_maps, a7, c7 = make_in_maps(x, W_leaf, weights, biases, w_out)
    nc = build_nc(float(b_out[0]), a7, c7)
    kw = dict(_run_kwargs or {})
    res = run_bass_kernel_spmd(nc, in_maps, core_ids=list(range(N_CORES)), **kw)
    out = gather_out(res.results)
    if _run_kwargs is not None:
        kernel.last_results = res
    return out
